# revision 6
# baseline (speedup 1.0000x reference)
"""Tensor-parallel multi-head attention for 8 Trainium2 NeuronCores.

Sharding (TP8 over heads): core c owns heads {2c, 2c+1} (128 q/k/v features)
for BOTH batch elements.  The context shards are exchanged with per-(batch,
seq-half) 8-core AllToAll collectives that convert head-sharding into
token-sharding (each core ends up with ALL 1024 context features for its
128-token slice), so the out-proj runs fully local with the full (row-
permuted) wo and writes its token slice of the output.  A2A moves 8x fewer
bytes than the AllGather alternative (~6.5us vs ~23us per exchange).

Per-core dataflow (activations kept transposed, [feature, token]):
  qT/kT/vT = W.T-chunks @ xT          (PE, bf16, fp32 PSUM accum)
  v        = PE-transpose(vT)          (with an appended ones-column)
  sT[k,q]  = kT-block.T @ qT           (both heads CONCURRENTLY: h0 uses PE
                                        row-groups 0-1, h1 rows 2-3 --
                                        tile_position auto-derives from the
                                        kT base partition, so the two 64-row
                                        matmuls overlap in the array)
  aT       = exp(sT/8 + mask_bias)     (ONE ACT call covers both heads'
                                        [128, 2, <=512] chunk; safe without
                                        max-subtraction: scores ~ N(0,1))
  ctxT;sum = [v|1].T @ aT              (per head; ones row gives the softmax
                                        denominator in psum row 64)
  ctxT    *= 1/sum                     (per q-block; the h0/h1 broadcast
                                        matmuls pair via PE col-groups)
  out[t,f] = sum_s cf_s.T @ woT_s      (post-A2A, per 128-token slice;
                                        cf chunks stationary, wo streaming,
                                        N=512 -- full-rate matmuls)

Scheduling: the attention stream is ACT(exp)-paced per step (~1.15us exp vs
~0.8us of PE per step), so all other PE work is threaded INTO it: batch-0
pass-0 starts as soon as the half-0 projections land; the remaining batch-0
projections ride inside pass 0, batch-1's projections inside batch-0's
pass 1, and the out-proj of each shipped chunk is inserted a safe margin
after its A2A completes (engine queues are FIFO, so a premature out-proj
matmul would block all attention work queued behind it).  Ctx matmuls lag
their scores by 2 steps so exp() has landed.  A tiny warm-up collective
absorbs the one-time mesh-entry barrier during the first projections.
Host side only reshapes/concatenates shards (dtype prep of inputs aside).
"""

import sys
from collections import deque

for _p in ("/opt/trn_rl_repo",):
    if _p not in sys.path:
        sys.path.append(_p)

import numpy as np
import ml_dtypes

import concourse.bass as bass  # noqa: F401
import concourse.mybir as mybir
import concourse.tile as tile
from concourse import bacc, bass_utils
from concourse.masks import make_identity, make_upper_triangular

BF16 = mybir.dt.bfloat16
F32 = mybir.dt.float32
F32R = mybir.dt.float32r
Exp = mybir.ActivationFunctionType.Exp

B, S, D = 2, 2048, 1024
T = B * S            # 4096 tokens across batches
H, DH = 16, 64
NCORES = 8
HPC = H // NCORES    # heads per core = 2
F = HPC * DH         # features per core = 128
KC = S // 128        # 16 k-chunks per batch
TPC = S // NCORES    # tokens per core per (batch, half) ship... (128)

_CACHED = {}


def _build(with_bias: bool):
    nc = bacc.Bacc(
        "TRN2",
        target_bir_lowering=False,
        debug=False,
        enable_asserts=True,
        num_devices=NCORES,
    )
    xT_d = nc.dram_tensor("xT", [D, T], BF16, kind="ExternalInput").ap()
    wqT_d = nc.dram_tensor("wqT", [D, F], BF16, kind="ExternalInput").ap()
    wkT_d = nc.dram_tensor("wkT", [D, F], BF16, kind="ExternalInput").ap()
    wvT_d = nc.dram_tensor("wvT", [D, F], BF16, kind="ExternalInput").ap()
    # full (input-permuted) out-proj weight: every core holds all of it
    woT_d = nc.dram_tensor("woT", [D, D], BF16, kind="ExternalInput").ap()
    b_d = {}
    if with_bias:
        for nm in ("bq", "bk", "bv"):
            b_d[nm] = nc.dram_tensor(nm, [1, F], BF16, kind="ExternalInput").ap()
        b_d["bo"] = nc.dram_tensor("bo", [1, D], BF16, kind="ExternalInput").ap()
    maskb_d = nc.dram_tensor("maskb", [128, B * KC], F32, kind="ExternalInput").ap()
    # [token-slice, (b,half), out-feature]
    outT_d = nc.dram_tensor("outT", [128, 2 * B, D], BF16, kind="ExternalOutput").ap()

    with tile.TileContext(nc) as tc:
        with (
            tc.tile_pool(name="singles", bufs=1) as sg,
            tc.tile_pool(name="att", bufs=6) as att_pool,
            tc.tile_pool(name="out", bufs=2) as out_pool,
            tc.tile_pool(name="cf", bufs=2) as cf_pool,
            tc.tile_pool(name="psA", bufs=2, space="PSUM") as psA,
            tc.tile_pool(name="psB", bufs=4, space="PSUM") as psB,
            tc.tile_pool(name="dram", bufs=1, space="DRAM") as dram,
        ):
            # ---- constants -------------------------------------------------
            ident = sg.tile([128, 128], BF16, name="ident")
            make_identity(nc, ident)
            trimask = sg.tile([128, 128], BF16, name="trimask")
            make_upper_triangular(nc, trimask, val=1.0, diag=True)
            ones64f = sg.tile([1, 64], F32, name="ones64f")
            nc.vector.memset(ones64f, 1.0)
            ones64r = sg.tile([1, 64], F32R, name="ones64r")
            nc.vector.tensor_copy(ones64r, ones64f)
            if with_bias:
                ones512 = sg.tile([1, 512], BF16, name="ones512")
                nc.vector.memset(ones512, 1.0)
                ones128 = sg.tile([1, 128], BF16, name="ones128")
                nc.vector.memset(ones128, 1.0)

            # ---- load inputs (split for early start) -----------------------
            maskb_sb = sg.tile([128, B * KC], F32, name="maskb_sb")
            nc.sync.dma_start(maskb_sb, maskb_d)
            w_sb = {}
            for nm, dd in (("v", wvT_d), ("k", wkT_d), ("q", wqT_d)):
                w_sb[nm] = sg.tile([128, 8, F], BF16, name=f"w{nm}T_sb")
                nc.sync.dma_start(w_sb[nm], dd.rearrange("(o p) f -> p o f", p=128))
            # full wo isn't needed until the first out-proj -- loaded after
            # the first x half
            w_sb["o"] = sg.tile([128, 8, D], BF16, name="woT_sb")
            # tiny warm-up collective: absorbs the one-time mesh-entry
            # barrier (~36us) while the projections run, so the first real
            # exchange isn't delayed by it
            warm_in = dram.tile([128, 8], BF16, name="warm_in")
            warm_out = dram.tile([128, 8], BF16, name="warm_out")
            nc.gpsimd.collective_compute(
                "AllToAll",
                mybir.AluOpType.bypass,
                replica_groups=[list(range(NCORES))],
                ins=[warm_in.opt()],
                outs=[warm_out.opt()],
            )
            b_sb = {}
            if with_bias:
                for nm in ("bq", "bk", "bv"):
                    b_sb[nm] = sg.tile([1, F], BF16, name=f"{nm}_sb")
                    nc.sync.dma_start(b_sb[nm], b_d[nm])
                b_sb["bo"] = sg.tile([1, D], BF16, name="bo_sb")
                nc.sync.dma_start(b_sb["bo"], b_d["bo"])

            # persistent tiles first; xT halves last (freed first: LIFO).
            qT_sb, qT_free = tc.tile([128, T], BF16, name="qT_sb")
            kT_sb, kT_free = tc.tile([128, T], BF16, name="kT_sb")
            ctxT_sb, ctxT_free = tc.tile([64, HPC, T], BF16, name="ctxT_sb")
            vT_sb, vT_free = tc.tile([128, T], BF16, name="vT_sb")
            xT_sb = {}
            xT_frees = []
            for b in (1, 0):
                xT_sb[b], f_ = tc.tile([128, 8, S], BF16, name=f"xT_sb{b}")
                xT_frees.append(f_)
            xT_r = xT_d.rearrange("(o p) f -> p o f", p=128)
            # ki-major per half so the ki-ordered projection consumes the
            # stream as it lands; batch-0 halves first.
            for half in range(4):
                for ki in range(8):
                    cs = (half % 2) * 1024
                    nc.sync.dma_start(
                        xT_sb[half // 2][:, ki, cs:cs + 1024],
                        xT_r[:, ki, half * 1024:half * 1024 + 1024],
                    )
                if half == 0:
                    nc.sync.dma_start(
                        w_sb["o"], woT_d.rearrange("(o p) f -> p o f", p=128)
                    )

            v_ones = sg.tile([128, B * KC, HPC, DH + 1], BF16, name="v_ones")
            nc.vector.memset(v_ones, 1.0)

            # ---- projection / transpose chunks (emitted piecemeal) ---------
            def project(which, half, nb):
                """One [128, 512] slice of one projection: 8 ki-chunks."""
                w = w_sb[which]
                dst = {"v": vT_sb, "k": kT_sb, "q": qT_sb}[which]
                ps = psA.tile(
                    [128, 512], F32, tag="work", name=f"p_{which}_{half}_{nb}"
                )
                cs = (half % 2) * 1024 + nb * 512
                for ki in range(8):
                    nc.tensor.matmul(
                        ps,
                        lhsT=w[:, ki, :],
                        rhs=xT_sb[half // 2][:, ki, cs:cs + 512],
                        start=(ki == 0),
                        stop=(ki == 7 and not with_bias),
                    )
                if with_bias:
                    nc.tensor.matmul(
                        ps,
                        lhsT=b_sb["b" + which][0:1, :],
                        rhs=ones512[0:1, :],
                        start=False,
                        stop=True,
                    )
                d0 = half * 1024 + nb * 512
                nc.vector.tensor_copy(dst[:, d0:d0 + 512], ps)

            def vt_chunk(tb0):
                """Transpose 8 vT 128-blocks into v_ones rows."""
                pt = psA.tile([128, 1024], BF16, tag="work", name=f"vt_{tb0}")
                for i in range(8):
                    tb = tb0 + i
                    nc.tensor.transpose(
                        pt[:, i * 128:i * 128 + 128],
                        vT_sb[:, tb * 128:tb * 128 + 128],
                        ident,
                    )
                for i in range(8):
                    tb = tb0 + i
                    for h in range(HPC):
                        nc.vector.tensor_copy(
                            v_ones[:, tb, h, 0:DH],
                            pt[:, i * 128 + h * 64:i * 128 + h * 64 + 64],
                        )

            # ---- attention -------------------------------------------------
            sums_r = sg.tile([1, 4 * 512], F32R, name="sums_r")
            rec_sb = sg.tile([64, 1024], F32, name="rec_sb")
            ctx_tiles = {}

            def normalize_pair(b, qb):
                """Both heads of one q-block."""
                t0 = b * S
                cps = [ctx_tiles.pop((h, qb)) for h in range(HPC)]
                for h in range(HPC):
                    so = (2 * h + (qb % 2)) * 512
                    nc.vector.tensor_copy(
                        sums_r[0:1, so:so + 512], cps[h][DH:DH + 1, :]
                    )
                bc = psA.tile([128, 1024], F32, tag="work", name=f"bc_{b}_{qb}")
                for h in range(HPC):
                    so = (2 * h + (qb % 2)) * 512
                    nc.tensor.matmul(
                        bc[0:64, 512 * h:512 * h + 512],
                        lhsT=ones64r[0:1, :],
                        rhs=sums_r[0:1, so:so + 512],
                        start=True,
                        stop=True,
                    )
                nc.vector.reciprocal_approx_fast(rec_sb, bc[0:64, :])
                for h in range(HPC):
                    nc.vector.tensor_mul(
                        ctxT_sb[:, h, t0 + qb * 512:t0 + qb * 512 + 512],
                        cps[h][0:DH, :],
                        rec_sb[:, 512 * h:512 * h + 512],
                    )

            def scores_pair(b, kc, c0, c1):
                """Both heads' score matmuls for one [c0,c1) q-chunk of one
                k-chunk (concurrent via PE row-groups) + ONE exp covering
                both; returns the [128, 2, w] attention-weights tile."""
                w = c1 - c0
                t0 = b * S
                q0 = kc * 128
                st = psA.tile(
                    [128, 2, 512], F32, tag="work", name=f"st_{b}_{kc}_{c0}"
                )
                for h in range(HPC):
                    po = 64 * h
                    nc.tensor.matmul(
                        st[:, h, 0:w],
                        lhsT=kT_sb[po:po + 64, t0 + q0:t0 + q0 + 128],
                        rhs=qT_sb[po:po + 64, t0 + c0:t0 + c1],
                        start=True,
                        stop=True,
                    )
                at = att_pool.tile([128, 2, 512], BF16, tag="att")
                nc.scalar.activation(
                    at[:, :, 0:w],
                    st[:, :, 0:w],
                    Exp,
                    bias=maskb_sb[:, b * KC + kc:b * KC + kc + 1],
                    scale=0.125,
                )
                if c0 <= q0 < c1:  # diagonal 128-block: causal interior
                    d0 = q0 - c0
                    for h in range(HPC):
                        nc.vector.tensor_mul(
                            at[:, h, d0:d0 + 128], at[:, h, d0:d0 + 128], trimask
                        )
                return at

            def ctx_pair(b, kc, c0, c1, cps, at):
                w = c1 - c0
                qb = c0 // 512
                for h in range(HPC):
                    nc.tensor.matmul(
                        cps[h][0:DH + 1, c0 - qb * 512:c1 - qb * 512],
                        lhsT=v_ones[:, b * KC + kc, h, :],
                        rhs=at[:, h, 0:w],
                        start=(kc == 0),
                        stop=(kc == 4 * qb + 3),
                    )
                if kc == 4 * qb + 3:
                    normalize_pair(b, qb)

            # ---- per-(batch, half) AllToAll exchanges ----------------------
            # cc buffers are [128, 1024] bf16; A2A shards on the row axis:
            # 16-row block j (32KB) carries my 128 ctx features x the j-th
            # 128-token sub-slice, laid out [feature(h*64+dh), token].
            cc_in = {}
            cc_out = {}
            for b in range(B):
                for half in range(2):
                    cc_in[(b, half)] = dram.tile(
                        [128, 1024], BF16, name=f"cci_{b}_{half}"
                    )
                    cc_out[(b, half)] = dram.tile(
                        [128, 1024], BF16, name=f"cco_{b}_{half}"
                    )

            ctxF = {}

            def ship(b, half):
                t0 = b * S + half * 1024
                key = (b, half)
                # dst block layout: row j*16+h*8+a, col b8*128+t
                #   -> linear j*16384 + (h*64 + a*8 + b8)*128 + t
                dst = cc_in[key].rearrange(
                    "(j h a) (b8 t) -> h (a b8) j t", j=8, h=2, a=8, b8=8
                )
                for h in range(HPC):
                    nc.sync.dma_start(
                        dst[h],
                        ctxT_sb[:, h, t0:t0 + 1024].rearrange(
                            "d (j t) -> d j t", j=8
                        ),
                    )
                nc.gpsimd.collective_compute(
                    "AllToAll",
                    mybir.AluOpType.bypass,
                    replica_groups=[list(range(NCORES))],
                    ins=[cc_in[key].opt()],
                    outs=[cc_out[key].opt()],
                )
                # post-A2A: block s = source-core s's 128 features for MY
                # 128-token slice -> SBUF [feature-within-s, s, token]
                cf = cf_pool.tile([128, 8, 128], BF16, tag="cf", name=f"cf_{b}_{half}")
                ctxF[key] = cf
                src = cc_out[key].rearrange(
                    "(s a) (b8 t) -> (a b8) s t", s=8, a=16, b8=8
                )
                nc.sync.dma_start(cf[:, 0:4, :], src[:, 0:4, :])
                nc.sync.dma_start(cf[:, 4:8, :], src[:, 4:8, :])

            def outproj(b, half):
                """Token-sharded out-proj: out[t, f] for my 128-token slice.
                cf chunks are stationary, wo streams at N=512."""
                cf = ctxF[(b, half)]
                ps = psA.tile([128, 1024], F32, tag="work", name=f"o_{b}_{half}")
                for s in range(8):
                    for nb in range(2):
                        nc.tensor.matmul(
                            ps[:, nb * 512:nb * 512 + 512],
                            lhsT=cf[:, s, :],
                            rhs=w_sb["o"][:, s, nb * 512:nb * 512 + 512],
                            start=(s == 0),
                            stop=(s == 7 and not with_bias),
                        )
                if with_bias:
                    for nb in range(2):
                        nc.tensor.matmul(
                            ps[:, nb * 512:nb * 512 + 512],
                            lhsT=ones128[0:1, :],
                            rhs=b_sb["bo"][0:1, nb * 512:nb * 512 + 512],
                            start=False,
                            stop=True,
                        )
                ot = out_pool.tile([128, 1024], BF16, tag="out")
                nc.vector.tensor_copy(ot, ps)
                nc.sync.dma_start(outT_d[:, 2 * b + half, :], ot)

            # ---- build the interleaved, software-pipelined stream ---------
            def get_ctx(h, qb):
                if (h, qb) not in ctx_tiles:
                    ctx_tiles[(h, qb)] = psB.tile(
                        [128, 512], F32, tag="ctx", name=f"cx_{h}_{qb}"
                    )
                return ctx_tiles[(h, qb)]

            def make_steps(b, pas):
                qlo, qhi = (0, 1024) if pas == 0 else (1024, 2048)
                kcs = range(8) if pas == 0 else range(KC)
                steps = []
                for kc in kcs:
                    c = max(kc * 128, qlo)
                    while c < qhi:
                        c2 = min(qhi, (c // 512 + 1) * 512)
                        steps.append((b, kc, c, c2))
                        c = c2
                return steps

            pending = deque()

            def flush(n=None):
                k = len(pending) if n is None else n
                for _ in range(k):
                    pending.popleft()()

            def do_step(arg):
                b, kc, c0, c1 = arg
                at = scores_pair(b, kc, c0, c1)
                cps = [get_ctx(h, c0 // 512) for h in range(HPC)]
                pending.append(
                    lambda b=b, kc=kc, c0=c0, c1=c1, cps=cps, at=at:
                    ctx_pair(b, kc, c0, c1, cps, at)
                )
                if len(pending) > 2:
                    flush(1)

            # lead-in: just enough of batch 0 for pass 0 (half-0 k/v/q)
            for which in ("v", "k", "q"):
                project(which, 0, 0)
                project(which, 0, 1)
                if which == "v":
                    vt_chunk(0)

            # batch-0 pass 0 (12 steps) with the rest of batch-0's prep
            # interleaved; pass 1 (28 steps) with batch-1's prep.
            b0_p0_misc = {
                1: lambda: project("v", 1, 0),
                3: lambda: project("v", 1, 1),
                5: lambda: vt_chunk(8),
                7: lambda: project("k", 1, 0),
                8: lambda: project("k", 1, 1),
                10: lambda: project("q", 1, 0),
                11: lambda: project("q", 1, 1),
            }
            for j, stp in enumerate(make_steps(0, 0), start=1):
                do_step(stp)
                if j in b0_p0_misc:
                    b0_p0_misc[j]()
            flush()
            ship(0, 0)

            b0_p1_misc = {
                1: lambda: project("v", 2, 0),
                3: lambda: project("v", 2, 1),
                5: lambda: project("v", 3, 0),
                7: lambda: project("v", 3, 1),
                9: lambda: vt_chunk(16),
                11: lambda: vt_chunk(24),
                13: lambda: project("k", 2, 0),
                15: lambda: project("k", 2, 1),
                17: lambda: outproj(0, 0),
                19: lambda: project("k", 3, 0),
                21: lambda: project("k", 3, 1),
                23: lambda: project("q", 2, 0),
                25: lambda: project("q", 2, 1),
                26: lambda: project("q", 3, 0),
                27: lambda: project("q", 3, 1),
            }
            for j, stp in enumerate(make_steps(0, 1), start=1):
                do_step(stp)
                if j in b0_p1_misc:
                    b0_p1_misc[j]()
            flush()
            ship(0, 1)
            xT_frees[1]()  # xT batch 0

            # batch-1 attention with batch-0 out-projs interleaved (placed
            # late enough that the corresponding A2A has completed).
            for j, stp in enumerate(make_steps(1, 0), start=1):
                do_step(stp)
                if j == 10:
                    outproj(0, 1)
            flush()
            ship(1, 0)

            for j, stp in enumerate(make_steps(1, 1), start=1):
                do_step(stp)
                if j == 10:
                    outproj(1, 0)
            flush()
            ship(1, 1)
            outproj(1, 1)

            xT_frees[0]()  # xT batch 1
            vT_free()
            ctxT_free()
            kT_free()
            qT_free()

    nc.compile()
    return nc


def _get_program(with_bias: bool = False):
    key = ("nc", with_bias)
    if key not in _CACHED:
        _CACHED[key] = _build(with_bias)
    return _CACHED[key]


# gathered feature order: [core r, local-head h, dh] -> global feature
# global head of (r, h) is 2r + h, so feature index = (2r + h) * DH + dh
_PERM = np.array(
    [(2 * r + h) * DH + dh for r in range(NCORES) for h in range(HPC) for dh in range(DH)]
)


def kernel(x, mask, wq, bq, wk, bk, wv, bv, wo, bo):
    x = np.asarray(x, dtype=np.float32)
    mask = np.asarray(mask)
    bf = ml_dtypes.bfloat16

    with_bias = any(np.any(np.asarray(bb)) for bb in (bq, bk, bv, bo))
    nc = _get_program(with_bias)

    # [feature, batch*seq] activations
    xT = np.ascontiguousarray(x.reshape(T, D).T).astype(bf)
    # -3.25: constant score shift (softmax-invariant) keeping exp() well
    # inside bf16 range; masked keys get -10000.
    maskb = np.ascontiguousarray(
        (np.where(np.asarray(mask).reshape(B * KC, 128), -10000.0, 0.0) - 3.25)
        .astype(np.float32)
        .T
    )
    # full out-proj weight [in-feature (permuted to the gathered order), out]
    woT_full = np.ascontiguousarray(np.asarray(wo).T[_PERM, :]).astype(bf)
    in_maps = []
    for c in range(NCORES):
        fs = slice(c * F, (c + 1) * F)
        m = {
            "xT": xT,
            "wqT": np.ascontiguousarray(np.asarray(wq)[fs, :].T).astype(bf),
            "wkT": np.ascontiguousarray(np.asarray(wk)[fs, :].T).astype(bf),
            "wvT": np.ascontiguousarray(np.asarray(wv)[fs, :].T).astype(bf),
            "woT": woT_full,
            "maskb": maskb,
        }
        if with_bias:
            m["bq"] = np.asarray(bq)[fs].astype(bf).reshape(1, F)
            m["bk"] = np.asarray(bk)[fs].astype(bf).reshape(1, F)
            m["bv"] = np.asarray(bv)[fs].astype(bf).reshape(1, F)
            m["bo"] = np.asarray(bo).astype(bf).reshape(1, D)
        in_maps.append(m)

    res = bass_utils.run_bass_kernel_spmd(
        nc, in_maps, core_ids=list(range(NCORES)), trace=False
    )
    _CACHED["last_results"] = res

    out = np.empty((B, S, D), dtype=np.float32)
    for c in range(NCORES):
        o = np.asarray(res.results[c]["outT"], dtype=np.float32)  # [128, 4, D]
        for b in range(B):
            for half in range(2):
                s0 = half * 1024 + c * 128
                out[b, s0:s0 + 128, :] = o[:, 2 * b + half, :]
    return out
